# revision 1
# baseline (speedup 1.0000x reference)
"""Trainium2 Bass kernel for EnhancedWikiGraphSAGE (4-layer mean-aggr GraphSAGE
+ LayerNorm + skip + multi-scale fusion MLP) on 8 NeuronCores.

Sharding: nodes are range-partitioned across 8 cores (6250/core, padded to
6272 = 49*128). Each core keeps a full replicated copy of the per-layer node
feature table in its DRAM (written via AllGather of per-core shards) and
aggregates neighbor features for its own nodes with batched ucode gathers
(InstDMAGatherAnt, 16 idx per ring descriptor) instead of per-row indirect
DMAs. dma_gather indices are int16, so the 50176-row table is addressed via
two half-windows (even rows / odd rows, the table being declared [TBL/2, 2H]);
a host-side greedy parity-balancing pass assigns each node's within-tile
position so every destination list splits near-evenly between windows,
minimizing slot padding. Nodes are degree-sorted per core so 128-node tiles
have nearly uniform degree. The final output is de-permuted/unpadded on host.
"""
import sys
for p in ('/opt/trn_rl_repo', '/root/.axon_site/_ro/trn_rl_repo'):
    if p not in sys.path:
        sys.path.insert(0, p)

import numpy as np
import concourse.bass as bass
import concourse.bacc as bacc
import concourse.mybir as mybir
import concourse.tile as tile
from concourse.tile import add_dep_helper
from concourse.masks import make_identity
from concourse.bass_utils import run_bass_kernel_spmd

F32 = mybir.dt.float32
BF16 = mybir.dt.bfloat16
I32 = mybir.dt.int32
I16 = mybir.dt.int16
AX = mybir.AxisListType
OP = mybir.AluOpType
ACTF = mybir.ActivationFunctionType

P = 128
LN_EPS = 1e-5


class Cfg:
    def __init__(self, n_nodes, n_edges, in_dim, hid, n_layers, n_cores):
        self.N = n_nodes
        self.E = n_edges
        self.IN = in_dim
        self.H = hid
        self.L = n_layers
        self.C = n_cores
        self.NPC = n_nodes // n_cores          # real nodes per core
        assert self.NPC * n_cores == n_nodes
        self.NT = (self.NPC + P - 1) // P      # tiles per core
        self.PADN = self.NT * P                # padded nodes per core
        self.TBL = self.C * self.PADN          # replicated table rows
        # input-dim chunks for the embedding GEMM
        self.IN_CHUNKS = []
        o = 0
        while o < in_dim:
            c = min(P, in_dim - o)
            self.IN_CHUNKS.append((o, c))
            o += c


def _parity_balance(cfg, deg, perms, src, dst):
    """Reorder nodes within their (core, tile) so that each destination's
    in-neighbor rows split near-evenly between even and odd table rows.
    Returns updated perms (list per core)."""
    C, NPC, NT, N = cfg.C, cfg.NPC, cfg.NT, cfg.N
    # tile of each node (fixed by degree sort)
    sortpos = np.empty(N, dtype=np.int64)
    for k in range(C):
        inv = np.empty(NPC, dtype=np.int64)
        inv[perms[k]] = np.arange(NPC)
        sortpos[k * NPC:(k + 1) * NPC] = inv
    tile_of = sortpos // P                      # local tile id
    core_of = np.arange(N) // NPC

    # CSR src -> dst
    order = np.argsort(src, kind='stable')
    d_sorted = dst[order]
    starts = np.searchsorted(src[order], np.arange(N + 1))

    # initial parity from current (degree-sorted) position
    parity = (sortpos % 2).astype(np.int8)
    cnt = np.zeros((N, 2), np.int64)            # per dst: assigned E/O in-neighbors
    np.add.at(cnt, (d_sorted, parity[np.repeat(np.arange(N), np.diff(starts))]), 1)

    # refinement: per-tile exact re-assignment (coordinate descent)
    lastlo = (NT - 1) * P
    n_last = NPC - lastlo
    evens_last = (n_last + 1) // 2
    for _round in range(4):
        for k in range(C):
            for t in range(NT):
                lo = t * P
                hi = min((t + 1) * P, NPC)
                gids = k * NPC + perms[k][lo:hi]
                nA = evens_last if t == NT - 1 else 64
                # remove members' contributions
                edl = [d_sorted[starts[s]:starts[s + 1]] for s in gids]
                for s, ds in zip(gids, edl):
                    cnt[ds, parity[s]] -= 1
                scores = np.array([(cnt[ds, 0] - cnt[ds, 1]).sum() if len(ds) else 0
                                   for ds in edl])
                order = np.argsort(scores, kind='stable')
                parity[gids[order[:nA]]] = 0
                parity[gids[order[nA:]]] = 1
                for s, ds in zip(gids, edl):
                    cnt[ds, parity[s]] += 1

    # materialize new positions: within each tile, even-parity nodes take the
    # even positions (in current order), odd-parity nodes the odd positions.
    new_perms = []
    for k in range(C):
        pk = perms[k]                           # position -> orig local id
        npk = np.empty_like(pk)
        for t in range(NT):
            lo = t * P
            hi = min((t + 1) * P, NPC)
            ids = pk[lo:hi]                     # orig local ids in this tile
            pars = parity[k * NPC + ids]
            ev = ids[pars == 0]
            od = ids[pars == 1]
            pos = np.arange(lo, hi)
            npk[pos[(pos % 2) == 0]] = ev
            npk[pos[(pos % 2) == 1]] = od
        new_perms.append(npk)
    return new_perms


def preprocess(cfg, x, edge_index):
    """Host-side integer/index preprocessing + sharding. Returns
    (per_core_inputs, meta)."""
    N, C, NPC, PADN, NT = cfg.N, cfg.C, cfg.NPC, cfg.PADN, cfg.NT
    src = edge_index[0].astype(np.int64)
    dst = edge_index[1].astype(np.int64)
    deg = np.bincount(dst, minlength=N).astype(np.int64)

    # per-core degree sort (stable, descending)
    perms = []
    for k in range(C):
        d = deg[k * NPC:(k + 1) * NPC]
        perms.append(np.argsort(-d, kind='stable'))
    # balance parities of table rows within tiles
    perms = _parity_balance(cfg, deg, perms, src, dst)

    sortpos = np.empty(N, dtype=np.int64)
    for k in range(C):
        inv = np.empty(NPC, dtype=np.int64)
        inv[perms[k]] = np.arange(NPC)
        sortpos[k * NPC:(k + 1) * NPC] = inv
    tbl_row = (np.arange(N) // NPC) * PADN + sortpos

    owner = dst // NPC
    d_sorted_pos = sortpos[dst]
    src_row = tbl_row[src]
    src_par = (src_row % 2).astype(np.int64)    # 0 = even window, 1 = odd

    deg_sorted = np.zeros((C, PADN), dtype=np.int64)
    for k in range(C):
        deg_sorted[k, :NPC] = deg[k * NPC:(k + 1) * NPC][perms[k]]
    D_t = deg_sorted.reshape(C, NT, P).max(axis=(0, 2)).astype(np.int64)
    D_t = np.maximum(D_t, 1)

    # per-window degree of each sorted node, and per-tile max counts
    degw = np.zeros((2, C, PADN), np.int64)
    for w in (0, 1):
        m = src_par == w
        dw = np.bincount(dst[m], minlength=N)
        for k in range(C):
            degw[w, k, :NPC] = dw[k * NPC:(k + 1) * NPC][perms[k]]
    DA_t = degw[0].reshape(C, NT, P).max(axis=(0, 2)).astype(np.int64)
    DB_t = degw[1].reshape(C, NT, P).max(axis=(0, 2)).astype(np.int64)
    DA_t = np.maximum(DA_t, 1)
    DB_t = np.maximum(DB_t, 1)
    offsA = np.concatenate([[0], np.cumsum(DA_t)])
    offsB = np.concatenate([[0], np.cumsum(DB_t)])
    TOTA, TOTB = int(offsA[-1]), int(offsB[-1])

    # pad indices: zero rows 6250 (even) / 6251 (odd) of core 0's shard
    padA = NPC // 2        # row NPC (even, zeroed) -> window idx NPC/2
    padB = NPC // 2        # row NPC+1 (odd, zeroed) -> (NPC+1-1)/2
    assert NPC % 2 == 0 and NPC + 1 < PADN

    # slot tables per window: slotW[k][p, offsW[t]+j] = window idx of j-th
    # W-parity in-neighbor of node (t, p), or the pad idx
    slotA = np.full((C, P, TOTA), padA, dtype=np.int32)
    slotB = np.full((C, P, TOTB), padB, dtype=np.int32)
    for w, slot, offs in ((0, slotA, offsA), (1, slotB, offsB)):
        m_all = src_par == w
        widx = src_row // 2                     # same formula both windows
        for k in range(C):
            m = m_all & (owner == k)
            tp = d_sorted_pos[m]
            sr = widx[m]
            o = np.argsort(tp, kind='stable')
            tp = tp[o]; sr = sr[o]
            boundaries = np.flatnonzero(np.diff(tp)) + 1
            starts = np.concatenate([[0], boundaries])
            run_ids = np.zeros(len(tp), dtype=np.int64)
            if len(starts) > 1:
                run_ids[starts[1:]] = 1
            grp = np.cumsum(run_ids)
            rank = np.arange(len(tp)) - starts[grp]
            t = tp // P
            pp = tp % P
            slot[k, pp, offs[t] + rank] = sr

    assert slotA.max() < 32768 and slotB.max() < 32768

    # wrapped int16 index stream: one call per (tile-pair, window).
    # call token i -> (partition i%128, column i//128); idx array wrapped in
    # 16 partitions (idxs[ch, s] = token s*16+ch), replicated x8.
    GROUPS = [(g * 2, min(g * 2 + 2, NT)) for g in range((NT + 1) // 2)]

    def wrap_call(slot, offs, t0, t1):
        cols = slot[:, :, offs[t0]:offs[t1]]            # [C, P, sumD]
        toks = cols.transpose(0, 2, 1).reshape(C, -1)   # [C, ntok] slot-major
        W = toks.shape[1] // 16
        wrapped = toks.reshape(C, W, 16).transpose(0, 2, 1)  # [C, 16, W]
        return np.tile(wrapped, (1, 8, 1))              # [C, 128, W]

    blocks = []
    calls = []      # per group: ((woffA, WA, nA), (woffB, WB, nB))
    woff = 0
    for (t0, t1) in GROUPS:
        sA = int(DA_t[t0:t1].sum())
        sB = int(DB_t[t0:t1].sum())
        blocks.append(wrap_call(slotA, offsA, t0, t1))
        blocks.append(wrap_call(slotB, offsB, t0, t1))
        calls.append(((woff, sA * 8, sA * 128),
                      (woff + sA * 8, sB * 8, sB * 128)))
        woff += (sA + sB) * 8
    idx16 = np.concatenate(blocks, axis=2).astype(np.int16)   # [C, 128, WTOT]
    WTOT = woff

    degflat = deg_sorted.reshape(C, NT, P).transpose(0, 2, 1).astype(np.float32)

    per_core = []
    for k in range(C):
        xk = x[k * NPC:(k + 1) * NPC][perms[k]]
        xpad = np.zeros((PADN, cfg.IN), dtype=np.float32)
        xpad[:NPC] = xk
        xT = np.ascontiguousarray(xpad.T)
        d = {"idx16": np.ascontiguousarray(idx16[k]), "degflat": degflat[k]}
        for ci, (o, c) in enumerate(cfg.IN_CHUNKS):
            d[f"xT{ci}"] = np.ascontiguousarray(xT[o:o + c])
        per_core.append(d)

    meta = {"D_t": [int(v) for v in D_t],
            "DA_t": [int(v) for v in DA_t], "DB_t": [int(v) for v in DB_t],
            "WTOT": WTOT, "calls": calls, "groups": GROUPS, "perms": perms}
    return per_core, meta


def rep_bias(b):
    return np.ascontiguousarray(np.tile(np.asarray(b, np.float32)[None, :], (P, 1)))


def build_common_inputs(cfg, emb_W, emb_b, lin_l_W, lin_l_b, lin_r_W,
                        ln_g, ln_b, fus_W1, fus_b1, fus_W2, fus_b2):
    H, L = cfg.H, cfg.L
    d = {
        "emb_W": np.asarray(emb_W, np.float32),
        "emb_b_rep": rep_bias(emb_b),
        "lin_l_W": np.asarray(lin_l_W, np.float32),
        "lin_r_W": np.asarray(lin_r_W, np.float32),
        "fus_W1": np.asarray(fus_W1, np.float32),
        "fus_W2": np.asarray(fus_W2, np.float32),
        "fus_b1_rep": rep_bias(fus_b1),
        "fus_b2_rep": rep_bias(fus_b2),
    }
    d["lin_l_b_rep"] = np.stack([rep_bias(lin_l_b[i]) for i in range(L)])
    d["ln_g_rep"] = np.stack([rep_bias(ln_g[i]) for i in range(L)])
    d["ln_b_rep"] = np.stack([rep_bias(ln_b[i]) for i in range(L)])
    return d


def build_program(cfg, meta):
    N, C, NT, PADN, TBL, H, L = cfg.N, cfg.C, cfg.NT, cfg.PADN, cfg.TBL, cfg.H, cfg.L
    D_t = meta["D_t"]
    DA_t, DB_t = meta["DA_t"], meta["DB_t"]
    calls, WTOT = meta["calls"], meta["WTOT"]
    GROUPS = meta["groups"]
    CGMAX = max(sum(DA_t[t0:t1]) + sum(DB_t[t0:t1]) for (t0, t1) in GROUPS)

    nc = bacc.Bacc("TRN2", target_bir_lowering=False, debug=False, num_devices=C,
                   num_swdge_queues=4)

    # ---- I/O ----
    idx16 = nc.declare_dram_parameter("idx16", [P, WTOT], I16, isOutput=False)
    degflat = nc.declare_dram_parameter("degflat", [P, NT], F32, isOutput=False)
    xTs = [nc.declare_dram_parameter(f"xT{ci}", [c, PADN], F32, isOutput=False)
           for ci, (o, c) in enumerate(cfg.IN_CHUNKS)]
    emb_W = nc.declare_dram_parameter("emb_W", [cfg.IN, H], F32, isOutput=False)
    emb_b_rep = nc.declare_dram_parameter("emb_b_rep", [P, H], F32, isOutput=False)
    lin_l_W = nc.declare_dram_parameter("lin_l_W", [L, H, H], F32, isOutput=False)
    lin_r_W = nc.declare_dram_parameter("lin_r_W", [L, H, H], F32, isOutput=False)
    lin_l_b_rep = nc.declare_dram_parameter("lin_l_b_rep", [L, P, H], F32, isOutput=False)
    ln_g_rep = nc.declare_dram_parameter("ln_g_rep", [L, P, H], F32, isOutput=False)
    ln_b_rep = nc.declare_dram_parameter("ln_b_rep", [L, P, H], F32, isOutput=False)
    fus_W1 = nc.declare_dram_parameter("fus_W1", [(L + 1) * H, H], F32, isOutput=False)
    fus_W2 = nc.declare_dram_parameter("fus_W2", [H, H], F32, isOutput=False)
    fus_b1_rep = nc.declare_dram_parameter("fus_b1_rep", [P, H], F32, isOutput=False)
    fus_b2_rep = nc.declare_dram_parameter("fus_b2_rep", [P, H], F32, isOutput=False)
    out = nc.declare_dram_parameter("out", [PADN, H], F32, isOutput=True)

    # per-layer replicated bf16 tables, declared as row-pairs [TBL/2, 2H] so
    # the even/odd windows for dma_gather are plain column views
    tables = [nc.dram_tensor(f"table{i}", [TBL // 2, 2 * H], BF16, addr_space="Shared")
              for i in range(L)]

    rg = [list(range(C))]

    with tile.TileContext(nc) as tc:
        with (
            tc.tile_pool(name="const", bufs=1) as cp,
            tc.tile_pool(name="persist", bufs=1) as pp,
            tc.tile_pool(name="gbuf", bufs=3) as gp,
            tc.tile_pool(name="work", bufs=3) as wp,
            tc.tile_pool(name="mini", bufs=3) as mp,
            tc.tile_pool(name="xt", bufs=3) as xp,
            tc.tile_pool(name="psum", bufs=2, space="PSUM") as ps,
            tc.tile_pool(name="dram", bufs=1, space="DRAM") as dp,
        ):
            # ---------- one-time loads ----------
            ident = cp.tile([P, P], F32)
            make_identity(nc, ident[:])
            idx_sb = cp.tile([P, WTOT], I16)
            nc.sync.dma_start(out=idx_sb[:], in_=idx16[:])
            invdeg = cp.tile([P, NT], F32)
            nc.sync.dma_start(out=invdeg[:], in_=degflat[:])
            nc.vector.tensor_scalar_max(out=invdeg[:], in0=invdeg[:], scalar1=1.0)
            nc.vector.reciprocal(out=invdeg[:], in_=invdeg[:])

            embW_sb = []
            for ci, (o, c) in enumerate(cfg.IN_CHUNKS):
                w = cp.tile([P, H], F32, tag=f"embW{ci}")
                nc.sync.dma_start(out=w[:c, :], in_=emb_W[o:o + c, :])
                embW_sb.append(w)
            embb_sb = cp.tile([P, H], F32)
            nc.sync.dma_start(out=embb_sb[:], in_=emb_b_rep[:])

            wl_sb, wr_sb, bl_sb, g_sb, bb_sb = [], [], [], [], []
            for i in range(L):
                for lst, src_t, tag in ((wl_sb, lin_l_W, "wl"), (wr_sb, lin_r_W, "wr"),
                                        (bl_sb, lin_l_b_rep, "bl"), (g_sb, ln_g_rep, "lg"),
                                        (bb_sb, ln_b_rep, "lb")):
                    t = cp.tile([P, H], F32, tag=f"{tag}{i}")
                    nc.sync.dma_start(out=t[:], in_=src_t[i])
                    lst.append(t)
            fw1_sb = []
            for cidx in range(L + 1):
                t32 = cp.tile([P, H], F32, tag=f"fw1f{cidx}")
                nc.sync.dma_start(out=t32[:], in_=fus_W1[cidx * H:(cidx + 1) * H, :])
                fw1_sb.append(t32)
            fw2_sb = cp.tile([P, H], F32)
            nc.sync.dma_start(out=fw2_sb[:], in_=fus_W2[:])
            fb1_sb = cp.tile([P, H], F32)
            nc.sync.dma_start(out=fb1_sb[:], in_=fus_b1_rep[:])
            fb2_sb = cp.tile([P, H], F32)
            nc.sync.dma_start(out=fb2_sb[:], in_=fus_b2_rep[:])

            # ---------- persistent state ----------
            zero_t = cp.tile([P, H], BF16, name="zero_t")
            nc.vector.memset(zero_t[:], 0.0)
            eps_t = cp.tile([P, 1], F32, name="eps_t")
            nc.vector.memset(eps_t[:], LN_EPS)
            h_cur = pp.tile([P, NT * H], F32)          # node-major current h
            hT_cur = pp.tile([P, NT * H], F32)         # feature-major current h
            fus_acc = pp.tile([P, NT * H], F32)        # accumulated multi @ fus_W1
            shards = [dp.tile([PADN, H], BF16, tag=f"shard{i}", name=f"shard{i}")
                      for i in range(L)]

            def ts(t):
                return slice(t * H, (t + 1) * H)

            def finish_tile(i_rep, t, z_src, shard):
                ps_tr = ps.tile([P, H], F32, tag="tr")
                nc.tensor.transpose(out=ps_tr[:], in_=z_src, identity=ident[:])
                nc.scalar.activation(out=hT_cur[:, ts(t)], in_=ps_tr[:], func=ACTF.Copy)
                ps_f = ps.tile([P, H], F32, tag="fus")
                nc.tensor.matmul(ps_f[:], lhsT=hT_cur[:, ts(t)],
                                 rhs=fw1_sb[i_rep][:], start=True, stop=True)
                if i_rep == 0:
                    nc.scalar.activation(out=fus_acc[:, ts(t)], in_=ps_f[:], func=ACTF.Copy)
                else:
                    nc.vector.tensor_add(out=fus_acc[:, ts(t)], in0=fus_acc[:, ts(t)],
                                         in1=ps_f[:])
                if shard is not None:
                    hbf = wp.tile([P, H], BF16, tag="hbf")
                    nc.scalar.activation(out=hbf[:], in_=z_src, func=ACTF.Copy)
                    nc.sync.dma_start(out=shard[t * P:(t + 1) * P, :], in_=hbf[:])

            # ---------- embedding ----------
            for t in range(NT):
                ps_z = ps.tile([P, H], F32, tag="z")
                for ci, (o, c) in enumerate(cfg.IN_CHUNKS):
                    xt = xp.tile([P, H], F32, tag="xt")
                    nc.sync.dma_start(out=xt[:c, :], in_=xTs[ci][:, t * P:(t + 1) * P])
                    nc.tensor.matmul(ps_z[:], lhsT=xt[:c, :], rhs=embW_sb[ci][:c, :],
                                     start=(ci == 0), stop=(ci == len(cfg.IN_CHUNKS) - 1))
                z = wp.tile([P, H], F32, tag="z_sb")
                nc.vector.tensor_add(out=z[:], in0=ps_z[:], in1=embb_sb[:])
                nc.scalar.activation(out=h_cur[:, ts(t)], in_=z[:], func=ACTF.Relu)
                finish_tile(0, t, h_cur[:, ts(t)], shards[0])

            if cfg.NPC < PADN:
                nc.sync.dma_start(out=shards[0][cfg.NPC:PADN, :],
                                  in_=zero_t[:PADN - cfg.NPC, :])
            ag_insts = []
            ag0 = nc.gpsimd.collective_compute(
                "AllGather", OP.bypass, replica_groups=rg,
                ins=[shards[0][:]], outs=[tables[0][:]])
            ag_insts.append(ag0)

            # ---------- GNN layers ----------
            for i in range(L):
                table = tables[i]
                even_view = table[:, 0:H]       # rows 2e
                odd_view = table[:, H:2 * H]    # rows 2o+1
                for gi_, (t0, t1) in enumerate(GROUPS):
                  sA = sum(DA_t[t0:t1])
                  sB = sum(DB_t[t0:t1])
                  (woffA, WA, nA), (woffB, WB, nB) = calls[gi_]
                  gbuf = gp.tile([P, CGMAX, H], BF16, tag="g")
                  giA = nc.gpsimd.dma_gather(
                      out_ap=gbuf[:, 0:sA, :], in_ap=even_view,
                      idxs_ap=idx_sb[:, woffA:woffA + WA],
                      num_idxs=nA, num_idxs_reg=nA, elem_size=H,
                      elem_step=2 * H, single_packet=False, queue_num=gi_ % 4)
                  add_dep_helper(giA.ins, ag_insts[i].ins,
                                 reason="gather after table allgather")
                  giB = nc.gpsimd.dma_gather(
                      out_ap=gbuf[:, sA:sA + sB, :], in_ap=odd_view,
                      idxs_ap=idx_sb[:, woffB:woffB + WB],
                      num_idxs=nB, num_idxs_reg=nB, elem_size=H,
                      elem_step=2 * H, single_packet=False, queue_num=(gi_ + 2) % 4)
                  add_dep_helper(giB.ins, ag_insts[i].ins,
                                 reason="gather after table allgather")
                  aoff = 0
                  boff = sA
                  for t in range(t0, t1):
                    DA, DB = DA_t[t], DB_t[t]
                    agg = wp.tile([P, H], F32, tag="agg")
                    nc.vector.tensor_reduce(
                        out=agg[:], in_=gbuf[:, aoff:aoff + DA, :].rearrange("p k d -> p d k"),
                        axis=AX.X, op=OP.add)
                    agg2 = wp.tile([P, H], F32, tag="agg2")
                    nc.vector.tensor_reduce(
                        out=agg2[:], in_=gbuf[:, boff:boff + DB, :].rearrange("p k d -> p d k"),
                        axis=AX.X, op=OP.add)
                    aoff += DA
                    boff += DB
                    nc.vector.tensor_add(out=agg[:], in0=agg[:], in1=agg2[:])
                    aggm = wp.tile([P, H], F32, tag="aggm")
                    nc.scalar.activation(out=aggm[:], in_=agg[:], func=ACTF.Copy,
                                         scale=invdeg[:, t:t + 1])
                    # aggT
                    ps_at = ps.tile([P, H], F32, tag="at")
                    nc.tensor.transpose(out=ps_at[:], in_=aggm[:], identity=ident[:])
                    aggT = wp.tile([P, H], F32, tag="aggT")
                    nc.scalar.activation(out=aggT[:], in_=ps_at[:], func=ACTF.Copy)
                    # z = agg @ Wl + h @ Wr
                    ps_z = ps.tile([P, H], F32, tag="z")
                    nc.tensor.matmul(ps_z[:], lhsT=aggT[:], rhs=wl_sb[i][:],
                                     start=True, stop=False)
                    nc.tensor.matmul(ps_z[:], lhsT=hT_cur[:, ts(t)], rhs=wr_sb[i][:],
                                     start=False, stop=True)
                    z = wp.tile([P, H], F32, tag="z_sb")
                    nc.vector.tensor_add(out=z[:], in0=ps_z[:], in1=bl_sb[i][:])
                    # LayerNorm stats via bn_stats/bn_aggr
                    bst = mp.tile([P, 6], F32, tag="bst")
                    nc.vector.bn_stats(out=bst[:], in_=z[:])
                    mini = mp.tile([P, 2], F32, tag="mini")
                    nc.vector.bn_aggr(out=mini[:], in_=bst[:])
                    sd = mp.tile([P, 2], F32, tag="sd")
                    nc.scalar.activation(out=sd[:, 0:1], in_=mini[:, 1:2],
                                         func=ACTF.Sqrt, bias=eps_t[:, 0:1])
                    nc.vector.reciprocal(out=sd[:, 1:2], in_=sd[:, 0:1])
                    nm = mp.tile([P, 2], F32, tag="nm")
                    nc.vector.tensor_tensor(out=nm[:, 0:1], in0=mini[:, 0:1],
                                            in1=sd[:, 1:2], op=OP.mult)
                    nc.vector.tensor_scalar_mul(out=nm[:, 1:2], in0=nm[:, 0:1],
                                                scalar1=-1.0)
                    y = wp.tile([P, H], F32, tag="y")
                    nc.scalar.activation(out=y[:], in_=z[:], func=ACTF.Identity,
                                         scale=sd[:, 1:2], bias=nm[:, 1:2])
                    nc.vector.tensor_tensor(out=y[:], in0=y[:], in1=g_sb[i][:], op=OP.mult)
                    nc.vector.tensor_tensor(out=y[:], in0=y[:], in1=bb_sb[i][:], op=OP.add)
                    if i > 0:
                        nc.vector.tensor_tensor(out=y[:], in0=y[:], in1=h_cur[:, ts(t)],
                                                op=OP.add)
                    nc.scalar.activation(out=h_cur[:, ts(t)], in_=y[:], func=ACTF.Relu)
                    finish_tile(i + 1, t, h_cur[:, ts(t)],
                                shards[i + 1] if i + 1 < L else None)
                if i + 1 < L:
                    if cfg.NPC < PADN:
                        nc.sync.dma_start(out=shards[i + 1][cfg.NPC:PADN, :],
                                          in_=zero_t[:PADN - cfg.NPC, :])
                    ag = nc.gpsimd.collective_compute(
                        "AllGather", OP.bypass, replica_groups=rg,
                        ins=[shards[i + 1][:]], outs=[tables[i + 1][:]])
                    ag_insts.append(ag)

            # ---------- fusion MLP ----------
            for t in range(NT):
                f1 = wp.tile([P, H], F32, tag="f1")
                nc.vector.tensor_add(out=f1[:], in0=fus_acc[:, ts(t)], in1=fb1_sb[:])
                nc.scalar.activation(out=f1[:], in_=f1[:], func=ACTF.Relu)
                ps_t = ps.tile([P, H], F32, tag="tr")
                nc.tensor.transpose(out=ps_t[:], in_=f1[:], identity=ident[:])
                f1T = wp.tile([P, H], F32, tag="f1T")
                nc.scalar.activation(out=f1T[:], in_=ps_t[:], func=ACTF.Copy)
                ps_o = ps.tile([P, H], F32, tag="z")
                nc.tensor.matmul(ps_o[:], lhsT=f1T[:], rhs=fw2_sb[:],
                                 start=True, stop=True)
                o = wp.tile([P, H], F32, tag="o")
                nc.vector.tensor_add(out=o[:], in0=ps_o[:], in1=fb2_sb[:])
                nc.sync.dma_start(out=out[t * P:(t + 1) * P, :], in_=o[:])
    return nc


# ---------------------------------------------------------------------------
_CACHE = {}


def run(cfg, inputs, sim=False):
    x = np.asarray(inputs["x"], np.float32)
    edge_index = np.asarray(inputs["edge_index"])
    per_core, meta = preprocess(cfg, x, edge_index)
    common = build_common_inputs(
        cfg, inputs["emb_W"], inputs["emb_b"], inputs["lin_l_W"], inputs["lin_l_b"],
        inputs["lin_r_W"], inputs["ln_g"], inputs["ln_b"], inputs["fus_W1"],
        inputs["fus_b1"], inputs["fus_W2"], inputs["fus_b2"])

    key = (cfg.N, cfg.E, tuple(meta["D_t"]), tuple(meta["DA_t"]), tuple(meta["DB_t"]))
    if key not in _CACHE:
        nc = build_program(cfg, meta)
        nc.compile()
        _CACHE[key] = nc
    nc = _CACHE[key]

    in_maps = [dict(common, **per_core[k]) for k in range(cfg.C)]
    if sim:
        from concourse.bass_interp import MultiCoreSim
        s = MultiCoreSim(nc, num_cores=cfg.C)
        for k in range(cfg.C):
            for name, arr in in_maps[k].items():
                s.cores[k].tensor(name)[:] = arr
        s.simulate()
        shard_outs = [np.array(s.cores[k].tensor("out")) for k in range(cfg.C)]
    else:
        res = run_bass_kernel_spmd(nc, in_maps, list(range(cfg.C)))
        shard_outs = [res.results[k]["out"] for k in range(cfg.C)]

    outp = np.empty((cfg.N, cfg.H), np.float32)
    for k in range(cfg.C):
        outp[k * cfg.NPC + meta["perms"][k]] = shard_outs[k][:cfg.NPC]
    return outp


def kernel(**inputs) -> np.ndarray:
    cfg = Cfg(n_nodes=50000, n_edges=800000, in_dim=300, hid=128, n_layers=4,
              n_cores=8)
    return run(cfg, inputs)

